# revision 5
# baseline (speedup 1.0000x reference)
"""3x3 valid cross-correlation (single channel) + bias on 8 NeuronCores.

All HBM traffic is fp16 (host casts x -> fp16, device stores fp16, host
upcasts), halving the memory-bound roofline vs fp32. Balanced 65-strip
layout: the 8190 output rows split into 65 strips of 126; each core takes
8 contiguous strips (1008 rows) plus 1/8 of the columns of the 65th strip,
so PE and DMA work are even across cores.

Device kernel per strip: banded stationary B_dj [128, 128] (126 live
columns padded to 128 to enable the compiler's fast weight load) with
B[m+di, m] = w[di, dj]; 3 fp16 matmuls (one per horizontal tap, rhs
shifted by dj columns) accumulate into one PSUM bank (fp32); DVE/ACT
alternate draining PSUM rows 0..125 + bias -> fp16 SBUF; DMA out.
Engine layout: input DMAs on the SP HWDGE ring (6-deep prefetch pool so
the front-loaded in-stream covers the mid-run HBM 50/50 in/out split),
steady-state output DMAs on the GpSimd SWDGE ring, consts on the ACT
ring, and the final strip's outputs on the (by then idle) SP ring in 8
chunks so the last HBM write completes early. The tiny tail strip runs
first, and six throwaway matmuls on a zeroed scratch tile pre-ramp the
PE p-state during the DMA fill so real work starts at full clock.

Measured: ~103.7 us/core (trace), vs 229.5 us for the staged fp32
baseline.
"""
import numpy as np

import concourse.mybir as mybir
from concourse.bacc import Bacc
from concourse import tile
from concourse.bass_utils import run_bass_kernel_spmd

H = W = 8192
KH = KW = 3
OH, OW = H - KH + 1, W - KW + 1          # 8190 x 8190
NCORES = 8
M_TILE = 126                              # output rows per strip
K_TILE = 128
N_TILE = 512                              # psum bank width in fp32
M_PAD = 128                               # stationary cols padded for FWL
NSTRIPS = 8                               # main strips per core (1008 rows)
BAND = NSTRIPS * M_TILE                   # 1008 output rows per core
IN_ROWS = BAND + KH - 1                   # 1010 input rows per core band
TAIL_W = 1026                             # tail-strip input cols per core
TAIL_OW = 1024                            # tail-strip output cols per core
MM_DT = mybir.dt.float16

_CACHE = {}


def _col_tiles(total):
    tiles = []
    c0 = 0
    while c0 < total:
        tiles.append((c0, min(N_TILE, total - c0)))
        c0 += N_TILE
    return tiles


def _build():
    nc = Bacc()
    xb = nc.dram_tensor("xb", [IN_ROWS, W], MM_DT, kind="ExternalInput")
    xb2 = nc.dram_tensor("xb2", [K_TILE, TAIL_W], MM_DT, kind="ExternalInput")
    bands = nc.dram_tensor("bands", [K_TILE, KW * M_PAD], MM_DT,
                           kind="ExternalInput")
    biasb = nc.dram_tensor("biasb", [K_TILE, 1], mybir.dt.float32,
                           kind="ExternalInput")
    yb = nc.dram_tensor("yb", [BAND, OW], MM_DT, kind="ExternalOutput")
    yb2 = nc.dram_tensor("yb2", [M_TILE, TAIL_OW], MM_DT,
                         kind="ExternalOutput")

    ctiles = _col_tiles(OW)

    with tile.TileContext(nc) as tc:
        with (
            tc.tile_pool(name="consts", bufs=1) as cpool,
            tc.tile_pool(name="xin", bufs=6) as xpool,
            tc.tile_pool(name="yout", bufs=4) as ypool,
            tc.tile_pool(name="pp", bufs=6, space="PSUM") as pp,
            tc.tile_pool(name="warmpp", bufs=1, space="PSUM") as warmpp,
        ):
            # PE p-state warmup: throwaway matmuls on a zeroed scratch tile
            # with no DMA deps, so the array runs at full clock (2.4 GHz
            # needs ~3 us of continuous execution) by the time the first
            # real operand lands at ~10 us. Results are never read.
            warm = cpool.tile([K_TILE, N_TILE], MM_DT, name="warm")
            nc.gpsimd.memset(warm, 0)
            wps = warmpp.tile([K_TILE, N_TILE], mybir.dt.float32, name="wps")
            for _ in range(6):
                nc.tensor.matmul(wps, warm[:, :K_TILE], warm[:, :N_TILE],
                                 start=True, stop=True)

            bands_t = cpool.tile([K_TILE, KW * M_PAD], MM_DT, name="bands_t")
            nc.scalar.dma_start(out=bands_t, in_=bands.ap())
            bias_t = cpool.tile([K_TILE, 1], mybir.dt.float32, name="bias_t")
            nc.scalar.dma_start(out=bias_t, in_=biasb.ap())

            # Tail strip first: tiny input, gets PE going early; its store
            # overlaps mid-stream instead of serializing at the end.
            xt2 = xpool.tile([K_TILE, TAIL_W], MM_DT, name="xt2", tag="xt")
            nc.sync.dma_start(out=xt2, in_=xb2.ap())
            yt2 = ypool.tile([M_TILE, TAIL_OW], MM_DT, name="yt2", tag="yt")
            for j, (c0, wdt) in enumerate(_col_tiles(TAIL_OW)):
                ps = pp.tile([M_PAD, N_TILE], mybir.dt.float32,
                             name="ps", tag="ps")
                for dj in range(KW):
                    nc.tensor.matmul(
                        ps[:M_PAD, :wdt],
                        bands_t[:, dj * M_PAD: dj * M_PAD + M_PAD],
                        xt2[:, c0 + dj: c0 + dj + wdt],
                        start=(dj == 0),
                        stop=(dj == KW - 1),
                    )
                if j % 2 == 0:
                    nc.vector.tensor_scalar_add(yt2[:, c0:c0 + wdt],
                                                ps[:M_TILE, :wdt],
                                                bias_t[:M_TILE, :])
                else:
                    nc.scalar.add(yt2[:, c0:c0 + wdt], ps[:M_TILE, :wdt],
                                  bias_t[:M_TILE, :])
            nc.gpsimd.dma_start(out=yb2.ap(), in_=yt2)

            for s in range(NSTRIPS):
                r0 = s * M_TILE
                xt = xpool.tile([K_TILE, W], MM_DT, name="xt", tag="xt")
                nx = 4 if s == 0 else 2
                xc = W // nx
                for k in range(nx):
                    lo, hi = k * xc, (k + 1) * xc
                    nc.sync.dma_start(out=xt[:, lo:hi],
                                      in_=xb.ap()[r0:r0 + K_TILE, lo:hi])
                yt = ypool.tile([M_TILE, OW], MM_DT, name="yt", tag="yt")
                for j, (c0, wdt) in enumerate(ctiles):
                    ps = pp.tile([M_PAD, N_TILE], mybir.dt.float32,
                                 name="ps", tag="ps")
                    for dj in range(KW):
                        nc.tensor.matmul(
                            ps[:M_PAD, :wdt],
                            bands_t[:, dj * M_PAD: dj * M_PAD + M_PAD],
                            xt[:, c0 + dj: c0 + dj + wdt],
                            start=(dj == 0),
                            stop=(dj == KW - 1),
                        )
                    if j % 2 == 0:
                        nc.vector.tensor_scalar_add(yt[:, c0:c0 + wdt],
                                                    ps[:M_TILE, :wdt],
                                                    bias_t[:M_TILE, :])
                    else:
                        nc.scalar.add(yt[:, c0:c0 + wdt], ps[:M_TILE, :wdt],
                                      bias_t[:M_TILE, :])
                last = s == NSTRIPS - 1
                ny = 8 if last else 2
                yeng = nc.sync if last else nc.gpsimd
                yc = OW // ny
                for k in range(ny):
                    lo = k * yc
                    hi = OW if k == ny - 1 else (k + 1) * yc
                    yeng.dma_start(out=yb.ap()[r0:r0 + M_TILE, lo:hi],
                                   in_=yt[:, lo:hi])
    nc.finalize()
    return nc


def _make_bands(weight: np.ndarray) -> np.ndarray:
    bands = np.zeros((K_TILE, KW * M_PAD), np.float16)
    m = np.arange(M_TILE)
    for dj in range(KW):
        for di in range(KH):
            bands[m + di, dj * M_PAD + m] = weight[di, dj]
    return bands


def _run(inputs: dict, trace: bool = False):
    x = np.asarray(inputs["x"], dtype=np.float32)
    weight = np.asarray(inputs["weight"], dtype=np.float32)
    bias = np.asarray(inputs["bias"], dtype=np.float32)

    if "nc" not in _CACHE:
        _CACHE["nc"] = _build()
    nc = _CACHE["nc"]

    x16 = x.astype(np.float16)
    bands = _make_bands(weight)
    biasb = np.full((K_TILE, 1), bias[0], np.float32)

    tail_r0 = OH - M_TILE                 # 8064
    in_maps = []
    for c in range(NCORES):
        r0 = c * BAND
        xb2 = np.zeros((K_TILE, TAIL_W), np.float16)
        c0 = c * TAIL_OW
        cols = min(TAIL_W, W - c0)
        xb2[:, :cols] = x16[tail_r0:tail_r0 + K_TILE, c0:c0 + cols]
        in_maps.append({
            "xb": np.ascontiguousarray(x16[r0:r0 + IN_ROWS]),
            "xb2": xb2,
            "bands": bands,
            "biasb": biasb,
        })

    res = run_bass_kernel_spmd(nc, in_maps, core_ids=list(range(NCORES)),
                               trace=trace)

    out = np.empty((OH, OW), np.float32)
    for c in range(NCORES):
        r0 = c * BAND
        out[r0:r0 + BAND] = res.results[c]["yb"].astype(np.float32)
        c0 = c * TAIL_OW
        take = min(TAIL_OW, OW - c0)
        out[tail_r0:, c0:c0 + take] = \
            res.results[c]["yb2"][:, :take].astype(np.float32)
    return out, res


def kernel(**inputs) -> np.ndarray:
    out, _ = _run(inputs, trace=False)
    return out
